# revision 10
# baseline (speedup 1.0000x reference)
"""DeepGCN (4-layer GCN, N=50000 nodes, E=800000 edges, D=128) on 8 Trainium2
NeuronCores via Bass/Tile.

v3 strategy (fp16 data path, own-shard overlap, trimmed gathers):
 - Permute nodes into 8 shards x 49 slots of 128, balancing in-degree.
 - spmm via A @ (x W) = (A x) W: SWDGE dma_gather fetches fp16 source rows
   (256B descriptors); per 128-edge chunk a one-hot-times-val matrix M
   (DVE, fp16) routes edges to destination offsets: zT += G^T M in PSUM.
 - Edges are split per slot into own/lo/hi streams: own-shard sources are
   gathered from a local XSELF table written during pass-2, so those
   gathers overlap the AllGather of the full table.
 - Each dma_gather passes the exact per-core index count in a register
   (idx streams are -1 padded); the Q7 descriptor-gen cost then tracks
   the real edge count instead of the padded chunk grid.
 - Layers 1..3 keep h row-major (pass2 = DVE sub + ACT relu + DVE add,
   residual in row space, table write needs no transpose).  PairNorm
   stats use accumulating PE matmuls against ones.  gc_b is dropped --
   PairNorm centering provably cancels a per-feature bias.
 - Layer 4 runs transposed (xnT) so fc_out consumes it directly; xn_row
   slots are transposed to xnT during layer 4's gather stream (hidden).
"""

import sys

sys.path.insert(0, "/opt/trn_rl_repo")

import numpy as np

import concourse.bacc as bacc
import concourse.mybir as mybir
import concourse.tile as tile
from concourse.bass_utils import run_bass_kernel_spmd
from concourse.library_config import mlp
from concourse.masks import make_identity

P = 128
NCORES = 8
N = 50000
D = 128
C = 40
L = 4
SLOTS = 49
NS = SLOTS * P
LO_LIMIT = 32768
MAXCH = 8  # max chunks (128 idxs each) per dma_gather call
EPS_BN = 1e-5
EPS_PN = 1e-6

F32 = mybir.dt.float32
F16 = mybir.dt.float16
I16 = mybir.dt.int16
I32 = mybir.dt.int32

TRACE = False
LAST_EXEC_NS = None

_nc_cache = {}


# ------------------------------------------------------------------ host prep

def _positions(edge_row):
    """Assign nodes to (core, slot, offset) balancing in-degree."""
    deg = np.bincount(edge_row, minlength=N)
    order = np.argsort(-deg, kind="stable")
    r = np.arange(N)
    rnd, pc = r // NCORES, r % NCORES
    core_of_rank = np.where(rnd % 2 == 0, pc, NCORES - 1 - pc)

    pos = np.empty(N, np.int64)
    for c in range(NCORES):
        nodes_c = order[core_of_rank == c]
        m = len(nodes_c)
        rr = np.arange(m)
        rnd2, ps_ = rr // SLOTS, rr % SLOTS
        slot = np.where(rnd2 % 2 == 0, ps_, SLOTS - 1 - ps_)
        off = rnd2
        pos[nodes_c] = c * NS + slot * P + off

    pos2node = np.full(NCORES * NS, -1, np.int64)
    pos2node[pos] = np.arange(N)
    return pos, pos2node


def _preprocess(edge_row, edge_col, edge_val):
    pos, pos2node = _positions(edge_row)
    pd = pos[edge_row]
    ps = pos[edge_col]
    core = pd // NS
    slotg = (pd % NS) // P
    doff = pd % P
    src_core = ps // NS
    own = (src_core == core).astype(np.int64)
    hi_r = ((ps >= LO_LIMIT) & (own == 0)).astype(np.int64)
    # stream id: 0=own, 1=lo-remote, 2=hi-remote
    stream = np.where(own == 1, 0, np.where(hi_r == 1, 2, 1))
    gi = np.where(own == 1, ps - core * NS,
                  np.where(stream == 2, ps - LO_LIMIT, ps)).astype(np.int64)

    key = (core * SLOTS + slotg) * 3 + stream
    cnt = np.bincount(key, minlength=NCORES * SLOTS * 3).reshape(
        NCORES, SLOTS, 3)
    K_OW = np.ceil(cnt[:, :, 0].max(axis=0) / P).astype(int)
    K_LO = np.ceil(cnt[:, :, 1].max(axis=0) / P).astype(int)
    K_HI = np.ceil(cnt[:, :, 2].max(axis=0) / P).astype(int)
    KS = [K_OW, K_LO, K_HI]

    # global chunk columns per slot: [own | lo | hi], slot-major
    base = np.zeros((SLOTS, 3), int)
    ctr = 0
    for s in range(SLOTS):
        for t in range(3):
            base[s, t] = ctr
            ctr += KS[t][s]
    TOT = ctr
    # per-stream idx chunk bases (each stream packed separately, slot-major)
    sb = [np.concatenate([[0], np.cumsum(K)[:-1]]) for K in KS]
    KT = [int(K.sum()) for K in KS]

    # call lists (must mirror the build enumeration exactly):
    # all own calls slot-major, then per slot lo calls + hi calls
    calls_own = []
    for s in range(SLOTS):
        for b0 in range(0, int(K_OW[s]), MAXCH):
            calls_own.append((s, b0, min(MAXCH, int(K_OW[s]) - b0)))
    calls_rem = []
    for s in range(SLOTS):
        for b0 in range(0, int(K_LO[s]), MAXCH):
            calls_rem.append((1, s, b0, min(MAXCH, int(K_LO[s]) - b0)))
        for b0 in range(0, int(K_HI[s]), MAXCH):
            calls_rem.append((2, s, b0, min(MAXCH, int(K_HI[s]) - b0)))
    NCALLS = len(calls_own) + len(calls_rem)

    per_core = []
    for c in range(NCORES):
        sel = np.flatnonzero(core == c)
        k = slotg[sel] * 3 + stream[sel]
        si = np.argsort(k, kind="stable")
        es = sel[si]
        ks = k[si]
        m = len(es)
        change = np.r_[True, np.diff(ks) != 0]
        segstart = np.maximum.accumulate(np.where(change, np.arange(m), 0))
        rank = np.arange(m) - segstart

        val_p = np.zeros((P, max(TOT, 1)), np.float32)
        dst_p = np.zeros((P, max(TOT, 1)), np.float32)
        flats = [np.full(max(KT[t], 1) * P, -1, np.int16) for t in range(3)]

        for t in range(3):
            msk = stream[es] == t
            ee = es[msk]
            rk = rank[msk]
            sl = slotg[ee]
            gch = base[sl, t] + rk // P
            val_p[rk % P, gch] = edge_val[ee]
            dst_p[rk % P, gch] = doff[ee]
            flats[t][sb[t][sl] * P + rk] = gi[ee]

        # per-call exact counts (ucode trims trailing -1 to this count)
        cnts = []
        for (s, b0, kk) in calls_own:
            cc = int(cnt[c, s, 0])
            cnts.append(int(np.clip(cc - b0 * P, 0, kk * P)))
        for (t, s, b0, kk) in calls_rem:
            cc = int(cnt[c, s, t])
            cnts.append(int(np.clip(cc - b0 * P, 0, kk * P)))
        gcnt = np.asarray(cnts, np.int32).reshape(1, NCALLS)

        def wrap(flat, kt):
            a = flat.reshape(kt * 8, 16).T  # [16, cols]
            return np.ascontiguousarray(np.tile(a, (8, 1)))

        per_core.append(dict(
            valp=val_p, dstp=dst_p,
            idx_ow=wrap(flats[0], max(KT[0], 1)),
            idx_lo=wrap(flats[1], max(KT[1], 1)),
            idx_hi=wrap(flats[2], max(KT[2], 1)),
            gcnt=gcnt,
        ))

    sched = tuple(tuple(int(x) for x in K) for K in KS)
    meta = dict(K_OW=K_OW, K_LO=K_LO, K_HI=K_HI, base=base, sb=sb,
                TOT=TOT, KT=KT, NCALLS=NCALLS,
                calls_own=calls_own, calls_rem=calls_rem)
    return pos, pos2node, per_core, sched, meta


# ------------------------------------------------------------------ bass build

def _build(meta):
    K_OW, K_LO, K_HI = meta["K_OW"], meta["K_LO"], meta["K_HI"]
    base, sb = meta["base"], meta["sb"]
    TOT, KT, NCALLS = meta["TOT"], meta["KT"], meta["NCALLS"]
    calls_own, calls_rem = meta["calls_own"], meta["calls_rem"]
    NTOT = NCORES * NS
    NGMAX = int((K_LO + K_HI).max())
    KOT = max(KT[0], 1)
    OP = mybir.AluOpType
    AF = mybir.ActivationFunctionType
    AX = mybir.AxisListType

    nc = bacc.Bacc("TRN2", target_bir_lowering=False, debug=False,
                   num_devices=NCORES)

    xt_own = nc.dram_tensor("xt_own", [P, NS], F32, kind="ExternalInput")
    idx_ow = nc.dram_tensor("idx_ow", [P, KOT * 8], I16,
                            kind="ExternalInput")
    idx_lo = nc.dram_tensor("idx_lo", [P, max(KT[1], 1) * 8], I16,
                            kind="ExternalInput")
    idx_hi = nc.dram_tensor("idx_hi", [P, max(KT[2], 1) * 8], I16,
                            kind="ExternalInput")
    valp = nc.dram_tensor("valp", [P, max(TOT, 1)], F32, kind="ExternalInput")
    dstp = nc.dram_tensor("dstp", [P, max(TOT, 1)], F32, kind="ExternalInput")
    gcnt = nc.dram_tensor("gcnt", [1, NCALLS], I32, kind="ExternalInput")
    fc_in_w = nc.dram_tensor("fc_in_w", [D, D], F32, kind="ExternalInput")
    fc_in_b = nc.dram_tensor("fc_in_b", [1, D], F32, kind="ExternalInput")
    bn_g = nc.dram_tensor("bn_g", [1, D], F32, kind="ExternalInput")
    bn_b = nc.dram_tensor("bn_b", [1, D], F32, kind="ExternalInput")
    gc_w = nc.dram_tensor("gc_w", [L * D, D], F32, kind="ExternalInput")
    fc_out_w = nc.dram_tensor("fc_out_w", [D, C], F32, kind="ExternalInput")
    fc_out_b = nc.dram_tensor("fc_out_b", [1, C], F32, kind="ExternalInput")
    out = nc.dram_tensor("out", [NS, C], F32, kind="ExternalOutput")

    RG = [list(range(NCORES))]

    with tile.TileContext(nc) as tc:
        nc.gpsimd.load_library(mlp)
        greg = nc.gpsimd.alloc_register("gcnt_reg")
        with (
            tc.tile_pool(name="const", bufs=1) as cp,
            tc.tile_pool(name="meta", bufs=1) as mp_,
            tc.tile_pool(name="big", bufs=1) as bp,
            tc.tile_pool(name="mpool", bufs=4) as mpl,
            tc.tile_pool(name="work", bufs=2) as wp,
            tc.tile_pool(name="small", bufs=1) as sp,
            tc.tile_pool(name="dram", bufs=1, space="DRAM") as dp,
        ):
            # ---------------- constants / inputs to SBUF
            iota_i = cp.tile([P, P], I32)
            iota_f = cp.tile([P, P], F32)
            nc.gpsimd.iota(iota_i[:], pattern=[[1, P]], base=0,
                           channel_multiplier=0)
            nc.vector.tensor_copy(iota_f[:], iota_i[:])
            ident = cp.tile([P, P], F32)
            make_identity(nc, ident[:])
            ident16 = cp.tile([P, P], F16)
            nc.vector.tensor_copy(ident16[:], ident[:])
            ones_col = cp.tile([P, 1], F32)
            nc.vector.memset(ones_col[:], 1.0)
            ones_col16 = cp.tile([P, 1], F16)
            nc.vector.memset(ones_col16[:], 1.0)
            ones_row = cp.tile([1, P], F32)
            nc.vector.memset(ones_row[:], 1.0)
            ones_row16 = cp.tile([1, P], F16)
            nc.vector.memset(ones_row16[:], 1.0)
            eps_bn_t = cp.tile([P, 1], F32)
            nc.vector.memset(eps_bn_t[:], EPS_BN)
            eps_pn_t = cp.tile([1, 1], F32)
            nc.vector.memset(eps_pn_t[:], EPS_PN)

            w1_raw = cp.tile([D, D], F32)
            nc.sync.dma_start(w1_raw[:], fc_in_w[:])
            fcb_s = cp.tile([1, D], F32)
            nc.sync.dma_start(fcb_s[:], fc_in_b[:])
            bn_s = cp.tile([2, D], F32)
            nc.sync.dma_start(bn_s[0:1, :], bn_g[:])
            nc.sync.dma_start(bn_s[1:2, :], bn_b[:])
            gw16 = [cp.tile([D, D], F16, tag=f"gw{i}", name=f"gw{i}")
                    for i in range(L)]
            for i in range(L):
                gw_raw = cp.tile([D, D], F32, tag=f"gwr{i}",
                                 name=f"gwr{i}")
                nc.sync.dma_start(gw_raw[:], gc_w[i * D:(i + 1) * D, :])
                nc.vector.tensor_copy(gw16[i][:], gw_raw[:])
            wo_raw = cp.tile([D, C], F32)
            nc.sync.dma_start(wo_raw[:], fc_out_w[:])
            wo16 = cp.tile([D, C], F16)
            nc.vector.tensor_copy(wo16[:], wo_raw[:])
            bo_raw = cp.tile([1, C], F32)
            nc.sync.dma_start(bo_raw[:], fc_out_b[:])
            bo16 = cp.tile([1, C], F16)
            nc.vector.tensor_copy(bo16[:], bo_raw[:])

            idx_ow_s = mp_.tile([P, KOT * 8], I16)
            nc.sync.dma_start(idx_ow_s[:], idx_ow[:])
            idx_lo_s = mp_.tile([P, max(KT[1], 1) * 8], I16)
            nc.sync.dma_start(idx_lo_s[:], idx_lo[:])
            idx_hi_s = mp_.tile([P, max(KT[2], 1) * 8], I16)
            nc.sync.dma_start(idx_hi_s[:], idx_hi[:])
            val_s = mp_.tile([P, max(TOT, 1)], F32)
            nc.sync.dma_start(val_s[:], valp[:])
            dst_s = mp_.tile([P, max(TOT, 1)], F32)
            nc.sync.dma_start(dst_s[:], dstp[:])
            gcnt_s = mp_.tile([1, NCALLS], I32)
            nc.sync.dma_start(gcnt_s[:], gcnt[:])

            # persistent per-slot state
            xnr = [bp.tile([P, P], F16, tag=f"xnr{s}", name=f"xnr{s}")
                   for s in range(SLOTS)]     # x row-major [dst, feat]
            xnT = [bp.tile([P, P], F16, tag=f"xnT{s}", name=f"xnT{s}")
                   for s in range(SLOTS)]     # x transposed (layer 4)
            hr16 = [bp.tile([P, P], F16, tag=f"hr{s}", name=f"hr{s}")
                    for s in range(SLOTS)]    # h of current layer

            # gather targets: own table chunks + double-buffered remote
            G_own = bp.tile([P, KOT, P], F16, name="G_own")
            G_ab = [bp.tile([P, max(NGMAX, 1), P], F16, tag=f"G{i}",
                            name=f"G{i}") for i in range(2)]
            nc.vector.memset(G_own[:], 0.0)
            nc.vector.memset(G_ab[0][:], 0.0)
            nc.vector.memset(G_ab[1][:], 0.0)

            # DRAM internals
            X_a = dp.tile([NTOT, P], F16)
            X_b = dp.tile([NTOT, P], F16)
            XSELF = dp.tile([NS, P], F16)
            ag_in = dp.tile([NS, P], F16)
            st_in = dp.tile([P, 2], F32)
            st_out = dp.tile([P, 2], F32)

            def own_gathers():
                ci = 0
                for (s, b0, kk) in calls_own:
                    c0 = int(sb[0][s]) + b0
                    nc.gpsimd.load(greg, gcnt_s[0:1, ci:ci + 1])
                    nc.gpsimd.dma_gather(
                        G_own[:, c0:c0 + kk, :], XSELF[:],
                        idx_ow_s[:, c0 * 8:(c0 + kk) * 8],
                        kk * P, greg, P)
                    ci += 1

            def rem_gathers(s, XIN, Gt):
                klo = int(K_LO[s])
                ci = len(calls_own)
                for (t, s2, b0, kk) in calls_rem:
                    if s2 != s:
                        ci += 1
                        continue
                    c0 = (int(sb[1][s]) + b0 if t == 1
                          else int(sb[2][s]) + b0)
                    gofs = b0 if t == 1 else klo + b0
                    src = XIN[:] if t == 1 else XIN[LO_LIMIT:, :]
                    idxs = idx_lo_s if t == 1 else idx_hi_s
                    nc.gpsimd.load(greg, gcnt_s[0:1, ci:ci + 1])
                    nc.gpsimd.dma_gather(
                        Gt[:, gofs:gofs + kk, :], src,
                        idxs[:, c0 * 8:(c0 + kk) * 8],
                        kk * P, greg, P)
                    ci += 1

            def spmm_chunks(s, Gt, zT):
                """zT += G^T M over own+lo+hi chunks of slot s."""
                kow, klo, khi = int(K_OW[s]), int(K_LO[s]), int(K_HI[s])
                nch = kow + klo + khi
                j = 0
                for t, k in ((0, kow), (1, klo), (2, khi)):
                    for jj in range(k):
                        col = int(base[s, t]) + jj
                        if t == 0:
                            gsl = G_own[:, int(sb[0][s]) + jj, :]
                        elif t == 1:
                            gsl = Gt[:, jj, :]
                        else:
                            gsl = Gt[:, klo + jj, :]
                        M = mpl.tile([P, P], F16, tag="M")
                        nc.vector.tensor_scalar(
                            out=M[:], in0=iota_f[:],
                            scalar1=dst_s[:, col:col + 1],
                            scalar2=val_s[:, col:col + 1],
                            op0=OP.is_equal, op1=OP.mult)
                        nc.tensor.matmul(zT[:], lhsT=gsl, rhs=M[:],
                                         start=(j == 0),
                                         stop=(j == nch - 1))
                        j += 1

            # ---------------- phase 0: BN stats + folded fc_in
            with (
                tc.tile_pool(name="p0psum", bufs=2, space="PSUM") as pp0,
                tc.tile_pool(name="p0sb", bufs=1) as sp0,
            ):
                xt_s = sp0.tile([P, NS], F32)
                nc.sync.dma_start(xt_s[:], xt_own[:])

                colsum_o = sp0.tile([P, 1], F32)
                sumsq_o = sp0.tile([P, 1], F32)
                scratch = sp0.tile([P, NS], F32)
                nc.vector.tensor_reduce(colsum_o[:], xt_s[:], axis=AX.X,
                                        op=OP.add)
                nc.scalar.activation(scratch[:], xt_s[:], AF.Square,
                                     accum_out=sumsq_o[:])
                st2 = sp0.tile([P, 2], F32)
                nc.vector.tensor_copy(st2[:, 0:1], colsum_o[:])
                nc.vector.tensor_copy(st2[:, 1:2], sumsq_o[:])
                nc.sync.dma_start(st_in[:], st2[:])
                nc.gpsimd.collective_compute(
                    "AllReduce", OP.add, replica_groups=RG,
                    ins=[st_in[:]], outs=[st_out[:]])
                stg = sp0.tile([P, 2], F32)
                nc.sync.dma_start(stg[:], st_out[:])

                mu = sp0.tile([P, 1], F32)
                nc.vector.tensor_scalar_mul(mu[:], stg[:, 0:1], 1.0 / N)
                msq = sp0.tile([P, 1], F32)
                nc.vector.tensor_scalar_mul(msq[:], stg[:, 1:2], 1.0 / N)
                mu2 = sp0.tile([P, 1], F32)
                nc.vector.tensor_tensor(mu2[:], mu[:], mu[:], op=OP.mult)
                var = sp0.tile([P, 1], F32)
                nc.vector.tensor_tensor(var[:], msq[:], mu2[:],
                                        op=OP.subtract)
                sd = sp0.tile([P, 1], F32)
                nc.scalar.activation(sd[:], var[:], AF.Sqrt,
                                     bias=eps_bn_t[:])
                rs = sp0.tile([P, 1], F32)
                nc.vector.reciprocal(rs[:], sd[:])

                bnT_ps = pp0.tile([P, 2], F32, space="PSUM", tag="pp0a")
                nc.tensor.transpose(bnT_ps[:], bn_s[:], ident[:2, :2])
                bnT = sp0.tile([P, 2], F32)
                nc.scalar.copy(bnT[:], bnT_ps[:])
                a_t = sp0.tile([P, 1], F32)
                nc.vector.tensor_tensor(a_t[:], bnT[:, 0:1], rs[:],
                                        op=OP.mult)
                t2 = sp0.tile([P, 1], F32)
                nc.vector.tensor_tensor(t2[:], mu[:], a_t[:], op=OP.mult)
                csh = sp0.tile([P, 1], F32)
                nc.vector.tensor_tensor(csh[:], bnT[:, 1:2], t2[:],
                                        op=OP.subtract)
                W1f16 = sp0.tile([D, D], F16)
                nc.scalar.activation(W1f16[:], w1_raw[:], AF.Copy,
                                     scale=a_t[:])
                bp_ps = pp0.tile([1, D], F32, space="PSUM", tag="pp0a")
                nc.tensor.matmul(bp_ps[:], lhsT=csh[:], rhs=w1_raw[:],
                                 start=True, stop=True)
                b1 = sp0.tile([1, D], F32)
                nc.scalar.copy(b1[:], bp_ps[:])
                nc.vector.tensor_tensor(b1[:], b1[:], fcb_s[:], op=OP.add)
                b1_16 = sp0.tile([1, D], F16)
                nc.vector.tensor_copy(b1_16[:], b1[:])
                xt16 = sp0.tile([P, NS], F16)
                nc.vector.tensor_copy(xt16[:], xt_s[:])

                with tc.tile_pool(name="p0g", bufs=3, space="PSUM") as ppg:
                    for s in range(SLOTS):
                        g_ps = ppg.tile([P, D], F32, space="PSUM", tag="g0")
                        nc.tensor.matmul(
                            g_ps[:], lhsT=xt16[:, s * P:(s + 1) * P],
                            rhs=W1f16[:], start=True, stop=False)
                        nc.tensor.matmul(g_ps[:], lhsT=ones_row16[:],
                                         rhs=b1_16[:], start=False, stop=True)
                        x0 = wp.tile([P, D], F16, tag="x0")
                        nc.scalar.copy(x0[:], g_ps[:])
                        nc.sync.dma_start(ag_in[s * P:(s + 1) * P, :], x0[:])
                        nc.sync.dma_start(XSELF[s * P:(s + 1) * P, :], x0[:])
                nc.gpsimd.collective_compute(
                    "AllGather", OP.bypass, replica_groups=RG,
                    ins=[ag_in[:]], outs=[X_a[:]])

            # ---------------- layers
            for li in range(L):
                XIN = X_a if li % 2 == 0 else X_b
                XOUT = X_b if li % 2 == 0 else X_a
                last = li == L - 1
                with (
                    tc.tile_pool(name=f"l{li}ps", bufs=2, space="PSUM") as lp,
                    tc.tile_pool(name=f"l{li}st", bufs=1, space="PSUM") as sps,
                ):
                    colsum_ps = sps.tile([P, 1], F32, space="PSUM",
                                         tag="colsum")
                    sumsq_ps = sps.tile([P, 1], F32, space="PSUM",
                                        tag="sumsq")
                    if last:
                        colacc = sp.tile([P, 1], F32, tag="colacc")
                        sqacc = sp.tile([P, 1], F32, tag="sqacc")
                        nc.vector.memset(colacc[:], 0.0)
                        nc.vector.memset(sqacc[:], 0.0)
                    # own-shard gathers first: overlap the AllGather
                    own_gathers()
                    for s in range(SLOTS):
                        Gt = G_ab[s % 2]
                        rem_gathers(s, XIN, Gt)
                        zT = lp.tile([P, P], F32, space="PSUM", tag="zT",
                                     bufs=2)
                        spmm_chunks(s, Gt, zT)
                        zs16 = wp.tile([P, P], F16, tag="zs")
                        nc.scalar.copy(zs16[:], zT[:])
                        if not last:
                            # h row-major: lhsT = zT (fin on partitions)
                            h_ps = lp.tile([P, P], F32, space="PSUM",
                                           tag="h", bufs=2)
                            nc.tensor.matmul(h_ps[:], lhsT=zs16[:],
                                             rhs=gw16[li][:],
                                             start=True, stop=True)
                            nc.scalar.copy(hr16[s][:], h_ps[:])
                            # stats: colsum/sumsq over nodes via PE
                            nc.tensor.matmul(
                                colsum_ps[:], lhsT=hr16[s][:],
                                rhs=ones_col16[:],
                                start=(s == 0), stop=(s == SLOTS - 1))
                            sq = wp.tile([P, P], F16, tag="sq")
                            nc.scalar.square(sq[:], hr16[s][:])
                            nc.tensor.matmul(
                                sumsq_ps[:], lhsT=sq[:], rhs=ones_col16[:],
                                start=(s == 0), stop=(s == SLOTS - 1))
                        else:
                            # layer 4 transposed: hT = W^T zT
                            h_ps = lp.tile([P, P], F32, space="PSUM",
                                           tag="h", bufs=2)
                            nc.tensor.matmul(h_ps[:], lhsT=gw16[li][:],
                                             rhs=zs16[:],
                                             start=True, stop=True)
                            nc.scalar.copy(hr16[s][:], h_ps[:])
                            red = wp.tile([P, 1], F32, tag="red")
                            nc.vector.tensor_reduce(red[:], hr16[s][:],
                                                    axis=AX.X, op=OP.add)
                            nc.vector.tensor_tensor(colacc[:], colacc[:],
                                                    red[:], op=OP.add)
                            sqs = wp.tile([P, P], F32, tag="sqs")
                            sqr = wp.tile([P, 1], F32, tag="sqr")
                            nc.scalar.activation(sqs[:], hr16[s][:],
                                                 AF.Square,
                                                 accum_out=sqr[:])
                            nc.vector.tensor_tensor(sqacc[:], sqacc[:],
                                                    sqr[:], op=OP.add)
                            # transpose previous-layer xnr into xnT (hidden)
                            tp_ps = lp.tile([P, P], F16, space="PSUM",
                                            tag="tp", bufs=2)
                            nc.tensor.transpose(tp_ps[:], xnr[s][:],
                                                ident16[:])
                            nc.scalar.copy(xnT[s][:], tp_ps[:])

                    # PairNorm stats -> AllReduce -> scalars
                    st2 = sp.tile([P, 2], F32, tag="st2")
                    if last:
                        nc.vector.tensor_copy(st2[:, 0:1], colacc[:])
                        nc.vector.tensor_copy(st2[:, 1:2], sqacc[:])
                    else:
                        nc.scalar.copy(st2[:, 0:1], colsum_ps[:])
                        nc.scalar.copy(st2[:, 1:2], sumsq_ps[:])
                    nc.sync.dma_start(st_in[:], st2[:])
                    nc.gpsimd.collective_compute(
                        "AllReduce", OP.add, replica_groups=RG,
                        ins=[st_in[:]], outs=[st_out[:]])
                    stg = sp.tile([P, 2], F32, tag="stg")
                    nc.sync.dma_start(stg[:], st_out[:])

                    cmean = sp.tile([P, 1], F32, tag="cmean")
                    nc.vector.tensor_scalar_mul(cmean[:], stg[:, 0:1],
                                                1.0 / N)
                    csq = sp.tile([P, 1], F32, tag="csq")
                    nc.vector.tensor_tensor(csq[:], stg[:, 0:1],
                                            stg[:, 0:1], op=OP.mult)
                    nc.vector.tensor_scalar_mul(csq[:], csq[:], 1.0 / N)
                    q = sp.tile([P, 1], F32, tag="q")
                    nc.vector.tensor_tensor(q[:], stg[:, 1:2], csq[:],
                                            op=OP.subtract)
                    tot_ps = lp.tile([1, 1], F32, space="PSUM", tag="h",
                                     bufs=2)
                    nc.tensor.matmul(tot_ps[:], lhsT=q[:], rhs=ones_col[:],
                                     start=True, stop=True)
                    tot_s = sp.tile([1, 1], F32, tag="tot")
                    nc.scalar.copy(tot_s[:], tot_ps[:])
                    rn = sp.tile([1, 1], F32, tag="rn")
                    nc.scalar.activation(rn[:], tot_s[:], AF.Sqrt,
                                         bias=eps_pn_t[:], scale=1.0 / N)
                    sres = sp.tile([1, 1], F32, tag="sres")
                    nc.vector.reciprocal(sres[:], rn[:])
                    sbc_ps = lp.tile([P, 1], F32, space="PSUM", tag="h",
                                     bufs=2)
                    nc.tensor.matmul(sbc_ps[:], lhsT=ones_row[:],
                                     rhs=sres[:], start=True, stop=True)
                    sbc = sp.tile([P, 1], F32, tag="sbc")
                    nc.scalar.copy(sbc[:], sbc_ps[:])

                    if not last:
                        # cmb[d, f] = cmean_f broadcast (row space)
                        cmb_ps = lp.tile([P, P], F32, space="PSUM",
                                         tag="zT", bufs=2)
                        nc.tensor.transpose(cmb_ps[:],
                                            cmean[:].to_broadcast([P, P]),
                                            ident[:])
                        cmb = sp.tile([P, P], F32, tag="cmb")
                        nc.scalar.copy(cmb[:], cmb_ps[:])
                        # pass 2 row-major: xnr = relu(s*(h - cmb)) + xnr_old
                        for s in range(SLOTS):
                            t32 = wp.tile([P, P], F32, tag="t32")
                            nc.vector.tensor_tensor(t32[:], hr16[s][:],
                                                    cmb[:], op=OP.subtract)
                            r16 = wp.tile([P, P], F16, tag="r16")
                            nc.scalar.activation(r16[:], t32[:], AF.Relu,
                                                 scale=sbc[:])
                            if li == 0:
                                nc.vector.tensor_copy(xnr[s][:], r16[:])
                            else:
                                nc.vector.tensor_tensor(xnr[s][:], xnr[s][:],
                                                        r16[:], op=OP.add)
                            nc.sync.dma_start(ag_in[s * P:(s + 1) * P, :],
                                              xnr[s][:])
                            nc.sync.dma_start(XSELF[s * P:(s + 1) * P, :],
                                              xnr[s][:])
                        nc.gpsimd.collective_compute(
                            "AllGather", OP.bypass, replica_groups=RG,
                            ins=[ag_in[:]], outs=[XOUT[:]])
                    else:
                        # transposed pass 2: xnT += relu(s*hT - s*cmean)
                        nbias = sp.tile([P, 1], F32, tag="nbias")
                        nc.vector.tensor_tensor(nbias[:], cmean[:], sbc[:],
                                                op=OP.mult)
                        nc.vector.tensor_scalar_mul(nbias[:], nbias[:], -1.0)
                        for s in range(SLOTS):
                            r16 = wp.tile([P, P], F16, tag="r16")
                            nc.scalar.activation(r16[:], hr16[s][:], AF.Relu,
                                                 bias=nbias[:], scale=sbc[:])
                            nc.vector.tensor_tensor(xnT[s][:], xnT[s][:],
                                                    r16[:], op=OP.add)

            # ---------------- fc_out (from xnT)
            with tc.tile_pool(name="fo", bufs=3, space="PSUM") as fp:
                for s in range(SLOTS):
                    o_ps = fp.tile([P, C], F32, space="PSUM", tag="o")
                    nc.tensor.matmul(o_ps[:], lhsT=xnT[s][:], rhs=wo16[:],
                                     start=True, stop=False)
                    nc.tensor.matmul(o_ps[:], lhsT=ones_row16[:],
                                     rhs=bo16[:], start=False, stop=True)
                    o_s = wp.tile([P, C], F32, tag="os")
                    nc.scalar.copy(o_s[:], o_ps[:])
                    nc.sync.dma_start(out[s * P:(s + 1) * P, :], o_s[:])

    nc.compile()
    return nc


# ------------------------------------------------------------------ kernel

def kernel(x, edge_row, edge_col, edge_val, bn_gamma, bn_beta,
           fc_in_w, fc_in_b, gc_w, gc_b, fc_out_w, fc_out_b):
    global LAST_EXEC_NS
    x = np.asarray(x, np.float32)
    edge_row = np.asarray(edge_row).astype(np.int64)
    edge_col = np.asarray(edge_col).astype(np.int64)
    edge_val = np.asarray(edge_val, np.float32)

    pos, pos2node, per_core, sched, meta = _preprocess(
        edge_row, edge_col, edge_val)

    if sched not in _nc_cache:
        _nc_cache[sched] = _build(meta)
    nc = _nc_cache[sched]

    x_pad = np.zeros((NCORES * NS, D), np.float32)
    x_pad[pos] = x
    shared = dict(
        fc_in_w=np.ascontiguousarray(fc_in_w, dtype=np.float32),
        fc_in_b=np.asarray(fc_in_b, np.float32).reshape(1, D),
        bn_g=np.asarray(bn_gamma, np.float32).reshape(1, D),
        bn_b=np.asarray(bn_beta, np.float32).reshape(1, D),
        gc_w=np.ascontiguousarray(
            np.asarray(gc_w, np.float32).reshape(L * D, D)),
        fc_out_w=np.ascontiguousarray(fc_out_w, dtype=np.float32),
        fc_out_b=np.asarray(fc_out_b, np.float32).reshape(1, C),
    )
    in_maps = []
    for c in range(NCORES):
        m = dict(shared)
        m["xt_own"] = np.ascontiguousarray(
            x_pad[c * NS:(c + 1) * NS].T)
        m.update(per_core[c])
        in_maps.append(m)

    res = run_bass_kernel_spmd(nc, in_maps, list(range(NCORES)),
                               trace=TRACE)
    LAST_EXEC_NS = res.exec_time_ns

    out_full = np.zeros((N, C), np.float32)
    for c in range(NCORES):
        rows = res.results[c]["out"]
        nodes = pos2node[c * NS:(c + 1) * NS]
        v = nodes >= 0
        out_full[nodes[v]] = rows[v]
    return out_full


# revision 12
# speedup vs baseline: 1.3424x; 1.3424x over previous
"""DeepGCN (4-layer GCN, N=50000 nodes, E=800000 edges, D=128) on 8 Trainium2
NeuronCores via Bass/Tile.

v4 strategy (fp16 data path, shared-output AllGather):
 - Permute nodes into 8 shards x 49 slots of 128, balancing in-degree.
 - spmm via A @ (x W) = (A x) W: SWDGE dma_gather fetches fp16 source rows
   (256B descriptors); per 128-edge chunk a one-hot-times-val matrix M
   (DVE, fp16) routes edges to destination offsets: zT += G^T M in PSUM.
 - Edges are split per slot into own/lo/hi streams: own-shard sources are
   gathered from a local XSELF table written during pass-2, so those
   gathers overlap the AllGather of the full table.
 - Each dma_gather passes the exact per-core index count in a register
   (idx streams are -1 padded); the Q7 descriptor-gen cost then tracks
   the real edge count instead of the padded chunk grid.
 - Layers 1..3 keep h row-major (pass2 = DVE sub + ACT relu + DVE add,
   residual in row space, table write needs no transpose).  PairNorm
   stats use accumulating PE matmuls against ones.  gc_b is dropped --
   PairNorm centering provably cancels a per-feature bias.
 - Layer 4 runs transposed (xnT) so fc_out consumes it directly; xn_row
   slots are transposed to xnT during layer 4's gather stream (hidden).
"""

import sys

sys.path.insert(0, "/opt/trn_rl_repo")

import numpy as np

import concourse.bacc as bacc
import concourse.mybir as mybir
import concourse.tile as tile
from concourse.bass_utils import run_bass_kernel_spmd
from concourse.library_config import mlp
from concourse.masks import make_identity

P = 128
NCORES = 8
N = 50000
D = 128
C = 40
L = 4
SLOTS = 49
NS = SLOTS * P
LO_LIMIT = 32768
MAXCH = 8  # max chunks (128 idxs each) per dma_gather call
EPS_BN = 1e-5
EPS_PN = 1e-6

F32 = mybir.dt.float32
F16 = mybir.dt.float16
I16 = mybir.dt.int16
I32 = mybir.dt.int32

TRACE = False
LAST_EXEC_NS = None

_nc_cache = {}


# ------------------------------------------------------------------ host prep

def _positions(edge_row):
    """Assign nodes to (core, slot, offset) balancing in-degree."""
    deg = np.bincount(edge_row, minlength=N)
    order = np.argsort(-deg, kind="stable")
    r = np.arange(N)
    rnd, pc = r // NCORES, r % NCORES
    core_of_rank = np.where(rnd % 2 == 0, pc, NCORES - 1 - pc)

    pos = np.empty(N, np.int64)
    for c in range(NCORES):
        nodes_c = order[core_of_rank == c]
        m = len(nodes_c)
        rr = np.arange(m)
        rnd2, ps_ = rr // SLOTS, rr % SLOTS
        slot = np.where(rnd2 % 2 == 0, ps_, SLOTS - 1 - ps_)
        off = rnd2
        pos[nodes_c] = c * NS + slot * P + off

    pos2node = np.full(NCORES * NS, -1, np.int64)
    pos2node[pos] = np.arange(N)
    return pos, pos2node


def _preprocess(edge_row, edge_col, edge_val):
    pos, pos2node = _positions(edge_row)
    pd = pos[edge_row]
    ps = pos[edge_col]
    core = pd // NS
    slotg = (pd % NS) // P
    doff = pd % P
    hi_r = (ps >= LO_LIMIT).astype(np.int64)
    # stream id: 0=lo, 1=hi
    stream = hi_r
    gi = (ps - hi_r * LO_LIMIT).astype(np.int64)

    key = (core * SLOTS + slotg) * 2 + stream
    cnt = np.bincount(key, minlength=NCORES * SLOTS * 2).reshape(
        NCORES, SLOTS, 2)
    K_LO = np.ceil(cnt[:, :, 0].max(axis=0) / P).astype(int)
    K_HI = np.ceil(cnt[:, :, 1].max(axis=0) / P).astype(int)
    KS = [K_LO, K_HI]

    # global chunk columns per slot: [lo | hi], slot-major
    base = np.zeros((SLOTS, 2), int)
    ctr = 0
    for s in range(SLOTS):
        for t in range(2):
            base[s, t] = ctr
            ctr += KS[t][s]
    TOT = ctr
    # per-stream idx chunk bases (each stream packed separately, slot-major)
    sb = [np.concatenate([[0], np.cumsum(K)[:-1]]) for K in KS]
    KT = [int(K.sum()) for K in KS]

    per_core = []
    for c in range(NCORES):
        sel = np.flatnonzero(core == c)
        k = slotg[sel] * 3 + stream[sel]
        si = np.argsort(k, kind="stable")
        es = sel[si]
        ks = k[si]
        m = len(es)
        change = np.r_[True, np.diff(ks) != 0]
        segstart = np.maximum.accumulate(np.where(change, np.arange(m), 0))
        rank = np.arange(m) - segstart

        val_p = np.zeros((P, max(TOT, 1)), np.float32)
        dst_p = np.zeros((P, max(TOT, 1)), np.float32)
        flats = [np.zeros(max(KT[t], 1) * P, np.int16) for t in range(2)]

        for t in range(2):
            msk = stream[es] == t
            ee = es[msk]
            rk = rank[msk]
            sl = slotg[ee]
            gch = base[sl, t] + rk // P
            val_p[rk % P, gch] = edge_val[ee]
            dst_p[rk % P, gch] = doff[ee]
            flats[t][sb[t][sl] * P + rk] = gi[ee]

        def wrap(flat, kt):
            a = flat.reshape(kt * 8, 16).T  # [16, cols]
            return np.ascontiguousarray(np.tile(a, (8, 1)))

        per_core.append(dict(
            valp=val_p, dstp=dst_p,
            idx_lo=wrap(flats[0], max(KT[0], 1)),
            idx_hi=wrap(flats[1], max(KT[1], 1)),
        ))

    sched = tuple(tuple(int(x) for x in K) for K in KS)
    meta = dict(K_LO=K_LO, K_HI=K_HI, base=base, sb=sb,
                TOT=TOT, KT=KT)
    return pos, pos2node, per_core, sched, meta


# ------------------------------------------------------------------ bass build

def _build(meta):
    K_LO, K_HI = meta["K_LO"], meta["K_HI"]
    base, sb = meta["base"], meta["sb"]
    TOT, KT = meta["TOT"], meta["KT"]
    NTOT = NCORES * NS
    NGMAX = int((K_LO + K_HI).max())
    OP = mybir.AluOpType
    AF = mybir.ActivationFunctionType
    AX = mybir.AxisListType

    nc = bacc.Bacc("TRN2", target_bir_lowering=False, debug=False,
                   num_devices=NCORES)

    xt_own = nc.dram_tensor("xt_own", [P, NS], F32, kind="ExternalInput")
    idx_lo = nc.dram_tensor("idx_lo", [P, max(KT[0], 1) * 8], I16,
                            kind="ExternalInput")
    idx_hi = nc.dram_tensor("idx_hi", [P, max(KT[1], 1) * 8], I16,
                            kind="ExternalInput")
    valp = nc.dram_tensor("valp", [P, max(TOT, 1)], F32, kind="ExternalInput")
    dstp = nc.dram_tensor("dstp", [P, max(TOT, 1)], F32, kind="ExternalInput")
    fc_in_w = nc.dram_tensor("fc_in_w", [D, D], F32, kind="ExternalInput")
    fc_in_b = nc.dram_tensor("fc_in_b", [1, D], F32, kind="ExternalInput")
    bn_g = nc.dram_tensor("bn_g", [1, D], F32, kind="ExternalInput")
    bn_b = nc.dram_tensor("bn_b", [1, D], F32, kind="ExternalInput")
    gc_w = nc.dram_tensor("gc_w", [L * D, D], F32, kind="ExternalInput")
    fc_out_w = nc.dram_tensor("fc_out_w", [D, C], F32, kind="ExternalInput")
    fc_out_b = nc.dram_tensor("fc_out_b", [1, C], F32, kind="ExternalInput")
    out = nc.dram_tensor("out", [NS, C], F32, kind="ExternalOutput")

    RG = [list(range(NCORES))]

    with tile.TileContext(nc) as tc:
        nc.gpsimd.load_library(mlp)
        with (
            tc.tile_pool(name="const", bufs=1) as cp,
            tc.tile_pool(name="meta", bufs=1) as mp_,
            tc.tile_pool(name="big", bufs=1) as bp,
            tc.tile_pool(name="mpool", bufs=4) as mpl,
            tc.tile_pool(name="work", bufs=2) as wp,
            tc.tile_pool(name="small", bufs=1) as sp,
            tc.tile_pool(name="dram", bufs=1, space="DRAM") as dp,
        ):
            # ---------------- constants / inputs to SBUF
            iota_i = cp.tile([P, P], I32)
            iota_f = cp.tile([P, P], F32)
            nc.gpsimd.iota(iota_i[:], pattern=[[1, P]], base=0,
                           channel_multiplier=0)
            nc.vector.tensor_copy(iota_f[:], iota_i[:])
            ident = cp.tile([P, P], F32)
            make_identity(nc, ident[:])
            ident16 = cp.tile([P, P], F16)
            nc.vector.tensor_copy(ident16[:], ident[:])
            ones_col = cp.tile([P, 1], F32)
            nc.vector.memset(ones_col[:], 1.0)
            ones_col16 = cp.tile([P, 1], F16)
            nc.vector.memset(ones_col16[:], 1.0)
            ones_row = cp.tile([1, P], F32)
            nc.vector.memset(ones_row[:], 1.0)
            ones_row16 = cp.tile([1, P], F16)
            nc.vector.memset(ones_row16[:], 1.0)
            eps_bn_t = cp.tile([P, 1], F32)
            nc.vector.memset(eps_bn_t[:], EPS_BN)
            eps_pn_t = cp.tile([1, 1], F32)
            nc.vector.memset(eps_pn_t[:], EPS_PN)

            w1_raw = cp.tile([D, D], F32)
            nc.sync.dma_start(w1_raw[:], fc_in_w[:])
            fcb_s = cp.tile([1, D], F32)
            nc.sync.dma_start(fcb_s[:], fc_in_b[:])
            bn_s = cp.tile([2, D], F32)
            nc.sync.dma_start(bn_s[0:1, :], bn_g[:])
            nc.sync.dma_start(bn_s[1:2, :], bn_b[:])
            gw16 = [cp.tile([D, D], F16, tag=f"gw{i}", name=f"gw{i}")
                    for i in range(L)]
            for i in range(L):
                gw_raw = cp.tile([D, D], F32, tag=f"gwr{i}",
                                 name=f"gwr{i}")
                nc.sync.dma_start(gw_raw[:], gc_w[i * D:(i + 1) * D, :])
                nc.vector.tensor_copy(gw16[i][:], gw_raw[:])
            wo_raw = cp.tile([D, C], F32)
            nc.sync.dma_start(wo_raw[:], fc_out_w[:])
            wo16 = cp.tile([D, C], F16)
            nc.vector.tensor_copy(wo16[:], wo_raw[:])
            bo_raw = cp.tile([1, C], F32)
            nc.sync.dma_start(bo_raw[:], fc_out_b[:])
            bo16 = cp.tile([1, C], F16)
            nc.vector.tensor_copy(bo16[:], bo_raw[:])

            idx_lo_s = mp_.tile([P, max(KT[0], 1) * 8], I16)
            nc.sync.dma_start(idx_lo_s[:], idx_lo[:])
            idx_hi_s = mp_.tile([P, max(KT[1], 1) * 8], I16)
            nc.sync.dma_start(idx_hi_s[:], idx_hi[:])
            val_s = mp_.tile([P, max(TOT, 1)], F32)
            nc.sync.dma_start(val_s[:], valp[:])
            dst_s = mp_.tile([P, max(TOT, 1)], F32)
            nc.sync.dma_start(dst_s[:], dstp[:])

            # persistent per-slot state
            xnr = [bp.tile([P, P], F16, tag=f"xnr{s}", name=f"xnr{s}")
                   for s in range(SLOTS)]     # x row-major [dst, feat]
            xnT = [bp.tile([P, P], F16, tag=f"xnT{s}", name=f"xnT{s}")
                   for s in range(SLOTS)]     # x transposed (layer 4)
            hr16 = [bp.tile([P, P], F16, tag=f"hr{s}", name=f"hr{s}")
                    for s in range(SLOTS)]    # h of current layer

            # double-buffered remote gather targets
            G_ab = [bp.tile([P, max(NGMAX, 1), P], F16, tag=f"G{i}",
                            name=f"G{i}") for i in range(2)]

            # DRAM internals (X tables Shared: fast HBM-HBM AllGather;
            # Shared is write-once -> one table per layer)
            X_t = [dp.tile([NTOT, P], F16, addr_space="Shared",
                           tag=f"X{i}", name=f"X{i}") for i in range(L)]
            ag_in = dp.tile([NS, P], F16)
            st_in = dp.tile([P, 2], F32)
            st_out = dp.tile([P, 2], F32)

            def rem_gathers(s, XIN, Gt):
                klo, khi = int(K_LO[s]), int(K_HI[s])
                for b0 in range(0, klo, MAXCH):
                    kk = min(MAXCH, klo - b0)
                    c0 = int(sb[0][s]) + b0
                    nc.gpsimd.dma_gather(
                        Gt[:, b0:b0 + kk, :], XIN[:],
                        idx_lo_s[:, c0 * 8:(c0 + kk) * 8],
                        kk * P, kk * P, P)
                for b0 in range(0, khi, MAXCH):
                    kk = min(MAXCH, khi - b0)
                    c0 = int(sb[1][s]) + b0
                    nc.gpsimd.dma_gather(
                        Gt[:, klo + b0:klo + b0 + kk, :],
                        XIN[LO_LIMIT:, :],
                        idx_hi_s[:, c0 * 8:(c0 + kk) * 8],
                        kk * P, kk * P, P)

            def spmm_chunks(s, Gt, zT):
                """zT += G^T M over lo+hi chunks of slot s."""
                klo, khi = int(K_LO[s]), int(K_HI[s])
                nch = klo + khi
                j = 0
                for t, k in ((0, klo), (1, khi)):
                    for jj in range(k):
                        col = int(base[s, t]) + jj
                        gsl = Gt[:, jj, :] if t == 0 else Gt[:, klo + jj, :]
                        M = mpl.tile([P, P], F16, tag="M")
                        nc.vector.tensor_scalar(
                            out=M[:], in0=iota_f[:],
                            scalar1=dst_s[:, col:col + 1],
                            scalar2=val_s[:, col:col + 1],
                            op0=OP.is_equal, op1=OP.mult)
                        nc.tensor.matmul(zT[:], lhsT=gsl, rhs=M[:],
                                         start=(j == 0),
                                         stop=(j == nch - 1))
                        j += 1

            # ---------------- phase 0: BN stats + folded fc_in
            with (
                tc.tile_pool(name="p0psum", bufs=2, space="PSUM") as pp0,
                tc.tile_pool(name="p0sb", bufs=1) as sp0,
            ):
                xt_s = sp0.tile([P, NS], F32)
                nc.sync.dma_start(xt_s[:], xt_own[:])

                colsum_o = sp0.tile([P, 1], F32)
                sumsq_o = sp0.tile([P, 1], F32)
                scratch = sp0.tile([P, NS], F32)
                nc.vector.tensor_reduce(colsum_o[:], xt_s[:], axis=AX.X,
                                        op=OP.add)
                nc.scalar.activation(scratch[:], xt_s[:], AF.Square,
                                     accum_out=sumsq_o[:])
                st2 = sp0.tile([P, 2], F32)
                nc.vector.tensor_copy(st2[:, 0:1], colsum_o[:])
                nc.vector.tensor_copy(st2[:, 1:2], sumsq_o[:])
                nc.sync.dma_start(st_in[:], st2[:])
                nc.gpsimd.collective_compute(
                    "AllReduce", OP.add, replica_groups=RG,
                    ins=[st_in[:]], outs=[st_out[:]])
                stg = sp0.tile([P, 2], F32)
                nc.sync.dma_start(stg[:], st_out[:])

                mu = sp0.tile([P, 1], F32)
                nc.vector.tensor_scalar_mul(mu[:], stg[:, 0:1], 1.0 / N)
                msq = sp0.tile([P, 1], F32)
                nc.vector.tensor_scalar_mul(msq[:], stg[:, 1:2], 1.0 / N)
                mu2 = sp0.tile([P, 1], F32)
                nc.vector.tensor_tensor(mu2[:], mu[:], mu[:], op=OP.mult)
                var = sp0.tile([P, 1], F32)
                nc.vector.tensor_tensor(var[:], msq[:], mu2[:],
                                        op=OP.subtract)
                sd = sp0.tile([P, 1], F32)
                nc.scalar.activation(sd[:], var[:], AF.Sqrt,
                                     bias=eps_bn_t[:])
                rs = sp0.tile([P, 1], F32)
                nc.vector.reciprocal(rs[:], sd[:])

                bnT_ps = pp0.tile([P, 2], F32, space="PSUM", tag="pp0a")
                nc.tensor.transpose(bnT_ps[:], bn_s[:], ident[:2, :2])
                bnT = sp0.tile([P, 2], F32)
                nc.scalar.copy(bnT[:], bnT_ps[:])
                a_t = sp0.tile([P, 1], F32)
                nc.vector.tensor_tensor(a_t[:], bnT[:, 0:1], rs[:],
                                        op=OP.mult)
                t2 = sp0.tile([P, 1], F32)
                nc.vector.tensor_tensor(t2[:], mu[:], a_t[:], op=OP.mult)
                csh = sp0.tile([P, 1], F32)
                nc.vector.tensor_tensor(csh[:], bnT[:, 1:2], t2[:],
                                        op=OP.subtract)
                W1f16 = sp0.tile([D, D], F16)
                nc.scalar.activation(W1f16[:], w1_raw[:], AF.Copy,
                                     scale=a_t[:])
                bp_ps = pp0.tile([1, D], F32, space="PSUM", tag="pp0a")
                nc.tensor.matmul(bp_ps[:], lhsT=csh[:], rhs=w1_raw[:],
                                 start=True, stop=True)
                b1 = sp0.tile([1, D], F32)
                nc.scalar.copy(b1[:], bp_ps[:])
                nc.vector.tensor_tensor(b1[:], b1[:], fcb_s[:], op=OP.add)
                b1_16 = sp0.tile([1, D], F16)
                nc.vector.tensor_copy(b1_16[:], b1[:])
                xt16 = sp0.tile([P, NS], F16)
                nc.vector.tensor_copy(xt16[:], xt_s[:])

                with tc.tile_pool(name="p0g", bufs=3, space="PSUM") as ppg:
                    for s in range(SLOTS):
                        g_ps = ppg.tile([P, D], F32, space="PSUM", tag="g0")
                        nc.tensor.matmul(
                            g_ps[:], lhsT=xt16[:, s * P:(s + 1) * P],
                            rhs=W1f16[:], start=True, stop=False)
                        nc.tensor.matmul(g_ps[:], lhsT=ones_row16[:],
                                         rhs=b1_16[:], start=False, stop=True)
                        x0 = wp.tile([P, D], F16, tag="x0")
                        nc.scalar.copy(x0[:], g_ps[:])
                        nc.sync.dma_start(ag_in[s * P:(s + 1) * P, :], x0[:])
                nc.gpsimd.collective_compute(
                    "AllGather", OP.bypass, replica_groups=RG,
                    ins=[ag_in[:]], outs=[X_t[0][:]])

            # ---------------- layers
            for li in range(L):
                XIN = X_t[li]
                last = li == L - 1
                with (
                    tc.tile_pool(name=f"l{li}ps", bufs=2, space="PSUM") as lp,
                    tc.tile_pool(name=f"l{li}st", bufs=1, space="PSUM") as sps,
                ):
                    colsum_ps = sps.tile([P, 1], F32, space="PSUM",
                                         tag="colsum")
                    sumsq_ps = sps.tile([P, 1], F32, space="PSUM",
                                        tag="sumsq")
                    if last:
                        colacc = sp.tile([P, 1], F32, tag="colacc")
                        sqacc = sp.tile([P, 1], F32, tag="sqacc")
                        nc.vector.memset(colacc[:], 0.0)
                        nc.vector.memset(sqacc[:], 0.0)
                    for s in range(SLOTS):
                        Gt = G_ab[s % 2]
                        rem_gathers(s, XIN, Gt)
                        zT = lp.tile([P, P], F32, space="PSUM", tag="zT",
                                     bufs=2)
                        spmm_chunks(s, Gt, zT)
                        zs16 = wp.tile([P, P], F16, tag="zs")
                        nc.scalar.copy(zs16[:], zT[:])
                        if not last:
                            # h row-major: lhsT = zT (fin on partitions)
                            h_ps = lp.tile([P, P], F32, space="PSUM",
                                           tag="h", bufs=2)
                            nc.tensor.matmul(h_ps[:], lhsT=zs16[:],
                                             rhs=gw16[li][:],
                                             start=True, stop=True)
                            nc.scalar.copy(hr16[s][:], h_ps[:])
                            # stats: colsum/sumsq over nodes via PE
                            nc.tensor.matmul(
                                colsum_ps[:], lhsT=hr16[s][:],
                                rhs=ones_col16[:],
                                start=(s == 0), stop=(s == SLOTS - 1))
                            sq = wp.tile([P, P], F16, tag="sq")
                            nc.scalar.square(sq[:], hr16[s][:])
                            nc.tensor.matmul(
                                sumsq_ps[:], lhsT=sq[:], rhs=ones_col16[:],
                                start=(s == 0), stop=(s == SLOTS - 1))
                        else:
                            # layer 4 transposed: hT = W^T zT
                            h_ps = lp.tile([P, P], F32, space="PSUM",
                                           tag="h", bufs=2)
                            nc.tensor.matmul(h_ps[:], lhsT=gw16[li][:],
                                             rhs=zs16[:],
                                             start=True, stop=True)
                            nc.scalar.copy(hr16[s][:], h_ps[:])
                            red = wp.tile([P, 1], F32, tag="red")
                            nc.vector.tensor_reduce(red[:], hr16[s][:],
                                                    axis=AX.X, op=OP.add)
                            nc.vector.tensor_tensor(colacc[:], colacc[:],
                                                    red[:], op=OP.add)
                            sqs = wp.tile([P, P], F32, tag="sqs")
                            sqr = wp.tile([P, 1], F32, tag="sqr")
                            nc.scalar.activation(sqs[:], hr16[s][:],
                                                 AF.Square,
                                                 accum_out=sqr[:])
                            nc.vector.tensor_tensor(sqacc[:], sqacc[:],
                                                    sqr[:], op=OP.add)
                            # transpose previous-layer xnr into xnT (hidden)
                            tp_ps = lp.tile([P, P], F16, space="PSUM",
                                            tag="tp", bufs=2)
                            nc.tensor.transpose(tp_ps[:], xnr[s][:],
                                                ident16[:])
                            nc.scalar.copy(xnT[s][:], tp_ps[:])

                    # PairNorm stats -> AllReduce -> scalars
                    st2 = sp.tile([P, 2], F32, tag="st2")
                    if last:
                        nc.vector.tensor_copy(st2[:, 0:1], colacc[:])
                        nc.vector.tensor_copy(st2[:, 1:2], sqacc[:])
                    else:
                        nc.scalar.copy(st2[:, 0:1], colsum_ps[:])
                        nc.scalar.copy(st2[:, 1:2], sumsq_ps[:])
                    nc.sync.dma_start(st_in[:], st2[:])
                    nc.gpsimd.collective_compute(
                        "AllReduce", OP.add, replica_groups=RG,
                        ins=[st_in[:]], outs=[st_out[:]])
                    stg = sp.tile([P, 2], F32, tag="stg")
                    nc.sync.dma_start(stg[:], st_out[:])

                    cmean = sp.tile([P, 1], F32, tag="cmean")
                    nc.vector.tensor_scalar_mul(cmean[:], stg[:, 0:1],
                                                1.0 / N)
                    csq = sp.tile([P, 1], F32, tag="csq")
                    nc.vector.tensor_tensor(csq[:], stg[:, 0:1],
                                            stg[:, 0:1], op=OP.mult)
                    nc.vector.tensor_scalar_mul(csq[:], csq[:], 1.0 / N)
                    q = sp.tile([P, 1], F32, tag="q")
                    nc.vector.tensor_tensor(q[:], stg[:, 1:2], csq[:],
                                            op=OP.subtract)
                    tot_ps = lp.tile([1, 1], F32, space="PSUM", tag="h",
                                     bufs=2)
                    nc.tensor.matmul(tot_ps[:], lhsT=q[:], rhs=ones_col[:],
                                     start=True, stop=True)
                    tot_s = sp.tile([1, 1], F32, tag="tot")
                    nc.scalar.copy(tot_s[:], tot_ps[:])
                    rn = sp.tile([1, 1], F32, tag="rn")
                    nc.scalar.activation(rn[:], tot_s[:], AF.Sqrt,
                                         bias=eps_pn_t[:], scale=1.0 / N)
                    sres = sp.tile([1, 1], F32, tag="sres")
                    nc.vector.reciprocal(sres[:], rn[:])
                    sbc_ps = lp.tile([P, 1], F32, space="PSUM", tag="h",
                                     bufs=2)
                    nc.tensor.matmul(sbc_ps[:], lhsT=ones_row[:],
                                     rhs=sres[:], start=True, stop=True)
                    sbc = sp.tile([P, 1], F32, tag="sbc")
                    nc.scalar.copy(sbc[:], sbc_ps[:])

                    if not last:
                        # cmb[d, f] = cmean_f broadcast (row space)
                        cmb_ps = lp.tile([P, P], F32, space="PSUM",
                                         tag="zT", bufs=2)
                        nc.tensor.transpose(cmb_ps[:],
                                            cmean[:].to_broadcast([P, P]),
                                            ident[:])
                        cmb = sp.tile([P, P], F32, tag="cmb")
                        nc.scalar.copy(cmb[:], cmb_ps[:])
                        # pass 2 row-major: xnr = relu(s*(h - cmb)) + xnr_old
                        for s in range(SLOTS):
                            t32 = wp.tile([P, P], F32, tag="t32")
                            nc.vector.tensor_tensor(t32[:], hr16[s][:],
                                                    cmb[:], op=OP.subtract)
                            r16 = wp.tile([P, P], F16, tag="r16")
                            nc.scalar.activation(r16[:], t32[:], AF.Relu,
                                                 scale=sbc[:])
                            if li == 0:
                                nc.vector.tensor_copy(xnr[s][:], r16[:])
                            else:
                                nc.vector.tensor_tensor(xnr[s][:], xnr[s][:],
                                                        r16[:], op=OP.add)
                            nc.sync.dma_start(ag_in[s * P:(s + 1) * P, :],
                                              xnr[s][:])
                        nc.gpsimd.collective_compute(
                            "AllGather", OP.bypass, replica_groups=RG,
                            ins=[ag_in[:]], outs=[X_t[li + 1][:]])
                    else:
                        # transposed pass 2: xnT += relu(s*hT - s*cmean)
                        nbias = sp.tile([P, 1], F32, tag="nbias")
                        nc.vector.tensor_tensor(nbias[:], cmean[:], sbc[:],
                                                op=OP.mult)
                        nc.vector.tensor_scalar_mul(nbias[:], nbias[:], -1.0)
                        for s in range(SLOTS):
                            r16 = wp.tile([P, P], F16, tag="r16")
                            nc.scalar.activation(r16[:], hr16[s][:], AF.Relu,
                                                 bias=nbias[:], scale=sbc[:])
                            nc.vector.tensor_tensor(xnT[s][:], xnT[s][:],
                                                    r16[:], op=OP.add)

            # ---------------- fc_out (from xnT)
            with tc.tile_pool(name="fo", bufs=3, space="PSUM") as fp:
                for s in range(SLOTS):
                    o_ps = fp.tile([P, C], F32, space="PSUM", tag="o")
                    nc.tensor.matmul(o_ps[:], lhsT=xnT[s][:], rhs=wo16[:],
                                     start=True, stop=False)
                    nc.tensor.matmul(o_ps[:], lhsT=ones_row16[:],
                                     rhs=bo16[:], start=False, stop=True)
                    o_s = wp.tile([P, C], F32, tag="os")
                    nc.scalar.copy(o_s[:], o_ps[:])
                    nc.sync.dma_start(out[s * P:(s + 1) * P, :], o_s[:])

    nc.compile()
    return nc


# ------------------------------------------------------------------ kernel

def kernel(x, edge_row, edge_col, edge_val, bn_gamma, bn_beta,
           fc_in_w, fc_in_b, gc_w, gc_b, fc_out_w, fc_out_b):
    global LAST_EXEC_NS
    x = np.asarray(x, np.float32)
    edge_row = np.asarray(edge_row).astype(np.int64)
    edge_col = np.asarray(edge_col).astype(np.int64)
    edge_val = np.asarray(edge_val, np.float32)

    pos, pos2node, per_core, sched, meta = _preprocess(
        edge_row, edge_col, edge_val)

    if sched not in _nc_cache:
        _nc_cache[sched] = _build(meta)
    nc = _nc_cache[sched]

    x_pad = np.zeros((NCORES * NS, D), np.float32)
    x_pad[pos] = x
    shared = dict(
        fc_in_w=np.ascontiguousarray(fc_in_w, dtype=np.float32),
        fc_in_b=np.asarray(fc_in_b, np.float32).reshape(1, D),
        bn_g=np.asarray(bn_gamma, np.float32).reshape(1, D),
        bn_b=np.asarray(bn_beta, np.float32).reshape(1, D),
        gc_w=np.ascontiguousarray(
            np.asarray(gc_w, np.float32).reshape(L * D, D)),
        fc_out_w=np.ascontiguousarray(fc_out_w, dtype=np.float32),
        fc_out_b=np.asarray(fc_out_b, np.float32).reshape(1, C),
    )
    in_maps = []
    for c in range(NCORES):
        m = dict(shared)
        m["xt_own"] = np.ascontiguousarray(
            x_pad[c * NS:(c + 1) * NS].T)
        m.update(per_core[c])
        in_maps.append(m)

    res = run_bass_kernel_spmd(nc, in_maps, list(range(NCORES)),
                               trace=TRACE)
    LAST_EXEC_NS = res.exec_time_ns

    out_full = np.zeros((N, C), np.float32)
    for c in range(NCORES):
        rows = res.results[c]["out"]
        nodes = pos2node[c * NS:(c + 1) * NS]
        v = nodes >= 0
        out_full[nodes[v]] = rows[v]
    return out_full
